# revision 17
# baseline (speedup 1.0000x reference)
"""BitNet attention (B=2, S=1024, H=4096, NH=32, NKV=8, HD=128) on 8 TRN2 cores.

Strategy (tensor-parallel over heads):
  - Host: quantize activations/weights to integer values (exact in bf16),
    de-interleave RoPE pairs via weight-row permutation, fold dequant scales
    into RoPE tables / per-token vectors.
  - Core c owns q-heads [4c, 4c+4), kv-head c, and o_proj output columns
    [512c, 512c+512).
  - Device: exact int QKV matmuls -> RoPE (DVE) -> scores^T (tk,tq) per
    (b,h) -> +mask -> exp (no max subtraction; scores are O(3)) -> softmax
    denominator via ones-matmul -> av^T feature-major (v transposed once on
    PE).  The softmax denominator and rmsnorm scale are never applied to the
    big tensors; they cancel into the int8-style quantizer and the final
    per-token output scale.  Tiny stats AllGather (16KB) -> quantize with
    exact round-half-to-even -> AllGather activations (bf16 ints) -> o_proj
    -> per-token scale -> out.
"""

import sys

if "/opt/trn_rl_repo" not in sys.path:
    sys.path.insert(0, "/opt/trn_rl_repo")

import numpy as np
import ml_dtypes

B, S, H = 2, 1024, 4096
NH, NKV, HD = 32, 8, 128
THETA = 500000.0
EPS = 1e-6
N_CORES = 8
T = B * S                    # 2048 tokens
QH = NH // N_CORES           # 4 q heads per core
OC = H // N_CORES            # 512 o_proj out-cols per core
ROUND_MAGIC = 12582912.0     # 1.5 * 2**23: (x + M) - M == rint(x) for |x| < 2**22

_PROGRAM = None  # (nc, core_ids) cache — the program is input-value independent


def _build_program():
    import concourse.bass as bass
    import concourse.tile as tile
    from concourse import mybir, bacc
    from concourse.masks import make_identity

    f32 = mybir.dt.float32
    bf16 = mybir.dt.bfloat16

    nc = bacc.Bacc("TRN2", target_bir_lowering=False, debug=False,
                   num_devices=N_CORES)

    # ---- I/O ----
    xT = nc.declare_dram_parameter("xT", [H, T], bf16, isOutput=False)
    wqkvT = nc.declare_dram_parameter("wqkvT", [H, QH * HD + 2 * HD], bf16, isOutput=False)
    woT = nc.declare_dram_parameter("woT", [H, OC], bf16, isOutput=False)
    ropeC = nc.declare_dram_parameter("ropeC", [HD, T], f32, isOutput=False)
    ropeS = nc.declare_dram_parameter("ropeS", [HD, T], f32, isOutput=False)
    maskT = nc.declare_dram_parameter("maskT", [128, S // 128, S], bf16, isOutput=False)
    vscale = nc.declare_dram_parameter("vscale", [128, T // 128], f32, isOutput=False)
    subln = nc.declare_dram_parameter("subln", [128, QH], f32, isOutput=False)
    swo127 = nc.declare_dram_parameter("swo127", [1, 1], f32, isOutput=False)
    out = nc.declare_dram_parameter("out", [T, OC], f32, isOutput=True)

    NT = T // 128        # 16 token tiles
    NK = H // 128        # 32 contraction chunks
    NQ = 4               # token quarters (512 tokens each)
    MQKV = QH + 2        # 6 output M-tiles in qkv projection

    from contextlib import ExitStack
    with tile.TileContext(nc) as tc:
        with ExitStack() as ctx:
            const = ctx.enter_context(tc.tile_pool(name="const", bufs=1))
            psum = ctx.enter_context(tc.tile_pool(name="psum", bufs=8, space="PSUM"))
            dram = ctx.enter_context(tc.tile_pool(name="dram", bufs=1, space="DRAM"))

            # ---- persistent SBUF ----
            ropeC_sb = const.tile([HD, T], f32)
            nc.sync.dma_start(out=ropeC_sb, in_=ropeC[:])
            ropeS_sb = const.tile([HD, T], f32)
            nc.sync.dma_start(out=ropeS_sb, in_=ropeS[:])
            maskT_sb = const.tile([128, S // 128, S], bf16)
            nc.sync.dma_start(out=maskT_sb, in_=maskT[:])
            vscale_sb = const.tile([128, NT], f32)
            nc.sync.dma_start(out=vscale_sb, in_=vscale[:])
            subln_sb = const.tile([128, QH], f32)
            nc.sync.dma_start(out=subln_sb, in_=subln[:])
            swo_sb = const.tile([1, 1], f32)
            nc.sync.dma_start(out=swo_sb, in_=swo127[:])
            swo_col = const.tile([128, 1], f32)
            nc.gpsimd.partition_broadcast(out_ap=swo_col, in_ap=swo_sb)
            ident = const.tile([128, 128], bf16)
            make_identity(nc, ident)
            ones_col = const.tile([128, 1], bf16)
            nc.vector.memset(ones_col, 1.0)

            wo_sb = const.tile([128, NK, OC], bf16)
            nc.sync.dma_start(out=wo_sb,
                              in_=woT[:].rearrange("(k p) m -> p k m", p=128))

            q_sb = const.tile([128, QH, T], bf16)
            k_sb = const.tile([128, T], bf16)
            vint_sb = const.tile([128, T], bf16)
            vtok_sb = const.tile([128, NT, HD], bf16)
            z_sb = const.tile([128, QH, T], bf16)
            d_tok = const.tile([128, QH, NT], f32)
            ss_tok = const.tile([128, QH, NT], f32)

            # ================= Phase A: QKV projection =================
            with ExitStack() as actx:
                wqkvp = actx.enter_context(tc.tile_pool(name="wqkvp", bufs=1))
                xpool = actx.enter_context(tc.tile_pool(name="xpool", bufs=4))
                rpool = actx.enter_context(tc.tile_pool(name="rpool", bufs=6))
                vintp = actx.enter_context(tc.tile_pool(name="vintp", bufs=1))

                wqkv_sb = wqkvp.tile([128, NK, MQKV * 128], bf16)
                nc.sync.dma_start(out=wqkv_sb,
                                  in_=wqkvT[:].rearrange("(k p) m -> p k m", p=128))
                vint_sb = vintp.tile([128, T], bf16)

                for quarter in range(NQ):
                    tq0 = quarter * 512
                    pq = [psum.tile([128, 512], f32, tag="bank", name=f"pq{m}")
                          for m in range(MQKV)]
                    for kk in range(NK):
                        xb = xpool.tile([128, 512], bf16, name="xb")
                        nc.sync.dma_start(out=xb, in_=xT[kk * 128:(kk + 1) * 128,
                                                         tq0:tq0 + 512])
                        for m in range(MQKV):
                            nc.tensor.matmul(pq[m][:],
                                             wqkv_sb[:, kk, m * 128:(m + 1) * 128],
                                             xb[:],
                                             start=(kk == 0), stop=(kk == NK - 1))
                    # rope q heads + k; copy v
                    for m in range(QH + 1):
                        m1 = rpool.tile([128, 512], bf16, name="m1")
                        nc.vector.tensor_mul(out=m1, in0=pq[m][:],
                                             in1=ropeC_sb[:, tq0:tq0 + 512])
                        m2 = rpool.tile([128, 512], bf16, name="m2")
                        nc.vector.tensor_mul(out=m2, in0=pq[m][:],
                                             in1=ropeS_sb[:, tq0:tq0 + 512])
                        m2s = rpool.tile([128, 512], bf16, name="m2s")
                        nc.sync.dma_start(out=m2s[0:64, :], in_=m2[64:128, :])
                        nc.sync.dma_start(out=m2s[64:128, :], in_=m2[0:64, :])
                        dst = (q_sb[:, m, tq0:tq0 + 512] if m < QH
                               else k_sb[:, tq0:tq0 + 512])
                        nc.gpsimd.tensor_add(out=dst, in0=m1[:], in1=m2s[:])
                    nc.vector.tensor_copy(out=vint_sb[:, tq0:tq0 + 512],
                                          in_=pq[QH + 1][:])

                # v transpose to token-major + dequant (per-token scale)
                for ti in range(NT):
                    pt = psum.tile([128, 128], bf16, tag="bank", name="pt")
                    nc.tensor.transpose(pt[:], vint_sb[:, ti * 128:(ti + 1) * 128],
                                        ident[:])
                    nc.scalar.activation(out=vtok_sb[:, ti, :], in_=pt[:],
                                         func=mybir.ActivationFunctionType.Copy,
                                         scale=vscale_sb[:, ti:ti + 1])

            # ================= Phase B: attention =================
            NB = S // 128  # 8 tk tiles per batch
            bctx = ExitStack()
            attnp = bctx.enter_context(tc.tile_pool(name="attnp", bufs=2))
            sqp = bctx.enter_context(tc.tile_pool(name="sqp", bufs=2))
            rowp = bctx.enter_context(tc.tile_pool(name="rowp", bufs=4))
            d_dram = dram.tile([QH, T], f32, name="d_dram")
            ss_dram = dram.tile([QH, T], f32, name="ss_dram")
            for b in range(B):
                for h in range(QH):
                    for chk in range(2):
                        tg0 = b * S + chk * 512   # global token offset (queries)
                        ts0 = chk * 512           # within-batch offset
                        attn = attnp.tile([128, NB, 512], bf16, name="attn")
                        for tk in range(NB):
                            ps = psum.tile([128, 512], f32, tag="bank", name="ps")
                            nc.tensor.matmul(
                                ps[:],
                                k_sb[:, b * S + tk * 128: b * S + (tk + 1) * 128],
                                q_sb[:, h, tg0:tg0 + 512],
                                start=True, stop=True)
                            nc.vector.tensor_add(out=ps[:], in0=ps[:],
                                                 in1=maskT_sb[:, tk, ts0:ts0 + 512])
                            nc.scalar.activation(out=attn[:, tk, :], in_=ps[:],
                                                 func=mybir.ActivationFunctionType.Exp)
                        pd = psum.tile([1, 512], f32, tag="bank", name="pd")
                        for tk in range(NB):
                            nc.tensor.matmul(pd[:], ones_col[:], attn[:, tk, :],
                                             start=(tk == 0), stop=(tk == NB - 1))
                        pav = psum.tile([128, 512], f32, tag="bank", name="pav")
                        for tk in range(NB):
                            nc.tensor.matmul(pav[:], vtok_sb[:, b * NB + tk, :],
                                             attn[:, tk, :],
                                             start=(tk == 0), stop=(tk == NB - 1))
                        nc.scalar.activation(out=z_sb[:, h, tg0:tg0 + 512], in_=pav[:],
                                             func=mybir.ActivationFunctionType.Copy,
                                             scale=subln_sb[:, h:h + 1])
                        sq = sqp.tile([128, 512], bf16, name="sq")
                        nc.scalar.activation(out=sq, in_=pav[:],
                                             func=mybir.ActivationFunctionType.Square)
                        pss = psum.tile([1, 512], f32, tag="bank", name="pss")
                        nc.tensor.matmul(pss[:], ones_col[:], sq[:],
                                         start=True, stop=True)
                        drow = rowp.tile([1, 512], f32, name="drow")
                        nc.vector.tensor_copy(out=drow, in_=pd[:])
                        ssrow = rowp.tile([1, 512], f32, name="ssrow")
                        nc.vector.tensor_copy(out=ssrow, in_=pss[:])
                        nc.sync.dma_start(out=d_dram[h, tg0:tg0 + 512], in_=drow[:])
                        nc.sync.dma_start(out=ss_dram[h, tg0:tg0 + 512], in_=ssrow[:])
            for h in range(QH):
                nc.sync.dma_start(
                    out=d_tok[:, h, :],
                    in_=d_dram[h].rearrange("(i p) -> p i", p=128))
                nc.sync.dma_start(
                    out=ss_tok[:, h, :],
                    in_=ss_dram[h].rearrange("(i p) -> p i", p=128))

            bctx.close()

            # ================= Phase C: stats + quant + o_proj =================
            cctx = ExitStack()
            treep = cctx.enter_context(tc.tile_pool(name="treep", bufs=1))
            browp = cctx.enter_context(tc.tile_pool(name="browp", bufs=1))
            bbp = cctx.enter_context(tc.tile_pool(name="bbp", bufs=2))
            zqp = cctx.enter_context(tc.tile_pool(name="zqp", bufs=2))
            lp = cctx.enter_context(tc.tile_pool(name="lp", bufs=3))
            outp = cctx.enter_context(tc.tile_pool(name="outp", bufs=3))

            # |z| max over each head's 128 partitions.  The HW verifier
            # requires equal base partitions for SB+SB tensor_tensor, so each
            # tree level first DMAs the upper half down to partition 0.
            tsc = treep.tile([64, QH, T], bf16, name="tsc")
            tup = treep.tile([64, QH, T], bf16, name="tup")
            nc.sync.dma_start(out=tup[:], in_=z_sb[64:128, :, :])
            nc.scalar.activation(out=tup[:], in_=tup[:],
                                 func=mybir.ActivationFunctionType.Abs)
            nc.scalar.activation(out=tsc[:], in_=z_sb[0:64, :, :],
                                 func=mybir.ActivationFunctionType.Abs)
            nc.vector.tensor_tensor(out=tsc[:], in0=tsc[:],
                                    in1=tup[:], op=mybir.AluOpType.max)
            w = 32
            while w >= 1:
                nc.sync.dma_start(out=tup[0:w, :, :], in_=tsc[w:2 * w, :, :])
                nc.vector.tensor_tensor(out=tsc[0:w, :, :], in0=tsc[0:w, :, :],
                                        in1=tup[0:w, :, :],
                                        op=mybir.AluOpType.max)
                w //= 2
            mz_dram = dram.tile([QH, T], bf16, name="mz_dram")
            for h in range(QH):
                nc.sync.dma_start(out=mz_dram[h, :], in_=tsc[0:1, h, :])
            mz_tok = const.tile([128, QH, NT], bf16)
            for h in range(QH):
                nc.sync.dma_start(
                    out=mz_tok[:, h, :],
                    in_=mz_dram[h].rearrange("(i p) -> p i", p=128))

            # local stats combine (token-major, FD=NT per head)
            dinv = const.tile([128, QH, NT], f32)
            nc.vector.reciprocal(out=dinv[:], in_=d_tok[:])
            dinv2 = const.tile([128, QH, NT], f32)
            nc.vector.tensor_mul(out=dinv2[:], in0=dinv[:], in1=dinv[:])
            ssn = const.tile([128, QH, NT], f32)
            nc.vector.tensor_mul(out=ssn[:], in0=ss_tok[:], in1=dinv2[:])
            mzn = const.tile([128, QH, NT], f32)
            nc.vector.tensor_mul(out=mzn[:], in0=mz_tok[:], in1=dinv[:])
            ss_loc = const.tile([128, NT], f32)
            nc.vector.tensor_add(out=ss_loc, in0=ssn[:, 0, :], in1=ssn[:, 1, :])
            nc.vector.tensor_add(out=ss_loc, in0=ss_loc, in1=ssn[:, 2, :])
            nc.vector.tensor_add(out=ss_loc, in0=ss_loc, in1=ssn[:, 3, :])
            mz_loc = const.tile([128, NT], f32)
            nc.vector.tensor_max(out=mz_loc, in0=mzn[:, 0, :], in1=mzn[:, 1, :])
            nc.vector.tensor_max(out=mz_loc, in0=mz_loc, in1=mzn[:, 2, :])
            nc.vector.tensor_max(out=mz_loc, in0=mz_loc, in1=mzn[:, 3, :])

            stats_dram = dram.tile([2, T], f32, name="stats_dram")
            nc.sync.dma_start(
                out=stats_dram[0].rearrange("(i p) -> p i", p=128),
                in_=ss_loc[:])
            nc.sync.dma_start(
                out=stats_dram[1].rearrange("(i p) -> p i", p=128),
                in_=mz_loc[:])
            gstats = dram.tile([2 * N_CORES, T], f32, name="gstats",
                               addr_space="Shared")
            nc.gpsimd.collective_compute(
                "AllGather", mybir.AluOpType.bypass,
                replica_groups=[list(range(N_CORES))],
                ins=[stats_dram[:].opt()], outs=[gstats[:].opt()])

            gss = const.tile([128, N_CORES, NT], f32)
            gmz = const.tile([128, N_CORES, NT], f32)
            for r in range(N_CORES):
                nc.sync.dma_start(
                    out=gss[:, r, :],
                    in_=gstats[2 * r].rearrange("(i p) -> p i", p=128))
                nc.sync.dma_start(
                    out=gmz[:, r, :],
                    in_=gstats[2 * r + 1].rearrange("(i p) -> p i", p=128))
            ss_tot_t = const.tile([128, NT], f32)
            nc.vector.tensor_add(out=ss_tot_t, in0=gss[:, 0, :], in1=gss[:, 1, :])
            for r in range(2, N_CORES):
                nc.vector.tensor_add(out=ss_tot_t, in0=ss_tot_t, in1=gss[:, r, :])
            m_tot_t = const.tile([128, NT], f32)
            nc.vector.tensor_max(out=m_tot_t, in0=gmz[:, 0, :], in1=gmz[:, 1, :])
            for r in range(2, N_CORES):
                nc.vector.tensor_max(out=m_tot_t, in0=m_tot_t, in1=gmz[:, r, :])
            ss_tot = ss_tot_t[:]
            m_tot = m_tot_t[:]

            # rms_inv = rsqrt(ss_tot/H + EPS), with one Newton refinement
            r0 = const.tile([128, NT], f32)
            nc.vector.tensor_scalar(out=r0, in0=ss_tot, scalar1=1.0 / H,
                                    scalar2=EPS, op0=mybir.AluOpType.mult,
                                    op1=mybir.AluOpType.add)
            sq0 = const.tile([128, NT], f32)
            nc.scalar.activation(out=sq0, in_=r0[:],
                                 func=mybir.ActivationFunctionType.Sqrt)
            y0 = const.tile([128, NT], f32)
            nc.vector.reciprocal(out=y0, in_=sq0[:])
            t1 = const.tile([128, NT], f32)
            nc.vector.tensor_mul(out=t1, in0=y0[:], in1=y0[:])
            nc.vector.tensor_mul(out=t1, in0=t1[:], in1=r0[:])
            nc.vector.tensor_scalar(out=t1, in0=t1[:], scalar1=-0.5, scalar2=1.5,
                                    op0=mybir.AluOpType.mult,
                                    op1=mybir.AluOpType.add)
            rms_inv = const.tile([128, NT], f32)
            nc.vector.tensor_mul(out=rms_inv, in0=y0[:], in1=t1[:])

            m_clip = const.tile([128, NT], f32)
            nc.vector.tensor_mul(out=m_clip, in0=m_tot, in1=rms_inv[:])
            nc.vector.tensor_scalar_max(out=m_clip, in0=m_clip[:], scalar1=1e-5)
            out_scale = const.tile([128, NT], f32)
            nc.vector.tensor_scalar_mul(out=out_scale, in0=m_clip[:],
                                        scalar1=swo_col[:])
            # gamma*rms = 127 * rms_inv / m_clip
            grms = const.tile([128, NT], f32)
            nc.vector.reciprocal(out=grms, in_=m_clip[:])
            nc.vector.tensor_mul(out=grms, in0=grms[:], in1=rms_inv[:])
            nc.vector.tensor_scalar_mul(out=grms, in0=grms[:], scalar1=127.0)

            # quantize z per head: zq = rint(z * (grms * dinv_h)) -> bf16 ints
            zq_dram = dram.tile([OC, T], bf16, name="zq_dram")
            b_dram = dram.tile([QH, T], f32, name="b_dram")
            for h in range(QH):
                bt = browp.tile([128, NT], f32, name="bt")
                nc.vector.tensor_mul(out=bt, in0=grms[:], in1=dinv[:, h, :])
                nc.sync.dma_start(
                    out=b_dram[h].rearrange("(i p) -> p i", p=128), in_=bt[:])
                brow = browp.tile([1, T], f32, name="brow")
                nc.sync.dma_start(out=brow[:], in_=b_dram[h])
                bb = bbp.tile([128, T], f32, name="bb")
                nc.gpsimd.partition_broadcast(out_ap=bb, in_ap=brow)
                zf = zqp.tile([128, T], f32, name="zf")
                nc.vector.tensor_mul(out=zf, in0=z_sb[:, h, :], in1=bb[:])
                zq = zqp.tile([128, T], bf16, name="zq")
                nc.vector.tensor_scalar(out=zq, in0=zf[:], scalar1=ROUND_MAGIC,
                                        scalar2=ROUND_MAGIC,
                                        op0=mybir.AluOpType.add,
                                        op1=mybir.AluOpType.subtract)
                nc.sync.dma_start(out=zq_dram[h * 128:(h + 1) * 128, :], in_=zq)

            zg = dram.tile([H, T], bf16, name="zg", addr_space="Shared")
            nc.gpsimd.collective_compute(
                "AllGather", mybir.AluOpType.bypass,
                replica_groups=[list(range(N_CORES))],
                ins=[zq_dram[:].opt()], outs=[zg[:].opt()])

            # o_proj: out[t, j] = sum_f zq[f, t] * wo[f, j], scaled per token
            for half in range(2):
                po = [psum.tile([128, OC], f32, tag="bank", name=f"po{half}_{tm}")
                      for tm in range(8)]
                for kk in range(NK):
                    lb = lp.tile([128, 1024], bf16, name="lb")
                    nc.sync.dma_start(
                        out=lb,
                        in_=zg[kk * 128:(kk + 1) * 128,
                               half * 1024:(half + 1) * 1024])
                    for tm in range(8):
                        nc.tensor.matmul(po[tm][:],
                                         lb[:, tm * 128:(tm + 1) * 128],
                                         wo_sb[:, kk, :],
                                         start=(kk == 0), stop=(kk == NK - 1))
                for tm in range(8):
                    tg = half * 8 + tm
                    osb = outp.tile([128, OC], f32, name="osb")
                    nc.scalar.activation(out=osb, in_=po[tm][:],
                                         func=mybir.ActivationFunctionType.Copy,
                                         scale=out_scale[:, tg:tg + 1])
                    nc.sync.dma_start(
                        out=out[tg * 128:(tg + 1) * 128, :], in_=osb)
            cctx.close()

    nc.compile()
    return nc


def _prep_inputs(hidden_states, attention_mask, w_q, w_k, w_v, w_o, subln_w):
    f32 = np.float32
    x = np.ascontiguousarray(hidden_states.reshape(T, H)).astype(f32, copy=False)
    amax = np.abs(x).max(axis=1)
    scale = (f32(127.0) / np.clip(amax, f32(1e-5), None)).astype(f32)
    xq = np.clip(np.round(x * scale[:, None]), -128.0, 127.0).astype(f32)
    sx_inv = (f32(1.0) / scale).astype(f32)
    xT_bf = np.ascontiguousarray(xq.T).astype(ml_dtypes.bfloat16)

    def wquant(w):
        s = f32(1.0) / np.clip(np.abs(w).mean(dtype=f32), f32(1e-5), None)
        wi = np.clip(np.round(w.astype(f32) * s), -1.0, 1.0).astype(f32)
        return wi, f32(1.0) / s

    wq_i, swq = wquant(w_q)
    wk_i, swk = wquant(w_k)
    wv_i, swv = wquant(w_v)
    wo_i, swo = wquant(w_o)

    # de-interleave rope pairs within each 128-row head block
    perm128 = np.concatenate([np.arange(0, 128, 2), np.arange(1, 128, 2)])

    # rope tables, with sqrt(swq*swk/sqrt(HD)) and per-token sx folded in
    inv_freq = (1.0 / (THETA ** (np.arange(0, HD, 2, dtype=np.float64) / HD))).astype(f32)
    pos = np.arange(S, dtype=f32)
    freqs = pos[:, None] * inv_freq[None, :]          # (S, 64)
    cosT = np.tile(np.cos(freqs).T.astype(f32), (1, B))   # (64, T)
    sinT = np.tile(np.sin(freqs).T.astype(f32), (1, B))
    rope_alpha = np.sqrt(swq * swk / np.sqrt(HD)).astype(f32)
    fold = (sx_inv[None, :] * rope_alpha).astype(f32)
    ropeC_np = np.concatenate([cosT, cosT], axis=0) * fold     # (128, T)
    ropeS_np = np.concatenate([sinT, -sinT], axis=0) * fold

    mask2d = np.asarray(attention_mask, dtype=f32)[0, 0]       # (S, S) [q, k]
    maskT_np = np.ascontiguousarray(
        mask2d.T.reshape(S // 128, 128, S).transpose(1, 0, 2)
    ).astype(ml_dtypes.bfloat16)                               # [p, i, q], tk=i*128+p

    vscale_np = np.ascontiguousarray(
        (sx_inv * swv).reshape(T // 128, 128).T).astype(f32)   # (128, NT)
    swo127_np = np.array([[swo / 127.0]], dtype=f32)

    in_maps = []
    for c in range(N_CORES):
        qrows = wq_i[c * 512:(c + 1) * 512]                     # (512, H)
        qrows = qrows.reshape(QH, 128, H)[:, perm128, :].reshape(QH * 128, H)
        krows = wk_i[c * 128:(c + 1) * 128][perm128]            # (128, H)
        vrows = wv_i[c * 128:(c + 1) * 128]                     # (128, H)
        wqkvT_c = np.ascontiguousarray(
            np.concatenate([qrows, krows, vrows], axis=0).T
        ).astype(ml_dtypes.bfloat16)                            # (H, 768)
        woT_c = np.ascontiguousarray(
            wo_i[c * 512:(c + 1) * 512].T).astype(ml_dtypes.bfloat16)  # (H, 512)
        subln_c = np.ascontiguousarray(
            np.asarray(subln_w, dtype=f32)[c * 512:(c + 1) * 512]
            .reshape(QH, 128).T).astype(f32)                    # (128, QH)
        in_maps.append({
            "xT": np.ascontiguousarray(xT_bf),
            "wqkvT": wqkvT_c,
            "woT": woT_c,
            "ropeC": np.ascontiguousarray(ropeC_np),
            "ropeS": np.ascontiguousarray(ropeS_np),
            "maskT": maskT_np,
            "vscale": vscale_np,
            "subln": subln_c,
            "swo127": swo127_np,
        })
    return in_maps


def kernel(**inputs):
    global _PROGRAM
    from concourse.bass_utils import run_bass_kernel_spmd

    if _PROGRAM is None:
        _PROGRAM = _build_program()
    nc = _PROGRAM

    in_maps = _prep_inputs(**inputs)
    res = run_bass_kernel_spmd(nc, in_maps, list(range(N_CORES)))
    cols = [res.results[c]["out"] for c in range(N_CORES)]
    full = np.concatenate(cols, axis=1).astype(np.float32)      # (T, H)
    return full.reshape(B, S, H)
